# revision 26
# baseline (speedup 1.0000x reference)
"""Bass/Trainium2 kernel for nn_ContrastiveAlignmentLoss.

reference math (B=256, N=512):
    global_sim = graph.mean(axis=(1, 2))                    # [B]
    sim        = outer(global_sim, global_sim)              # [B, B]
    same       = labels[:, None] == labels[None, :]
    pair_loss  = where(same, relu(0.5 - sim), relu(sim - 0.5))
    loss       = sum(triu(pair_loss, k=1)) / (B*(B-1)/2)

Distribution: data-parallel over B across 8 NeuronCores. Each core
sum-pools its 32 relation graphs (the memory-bound part: 32 MiB/core),
AllGathers the tiny [B] raw sums, and computes the pairwise loss
replicated (the 1/N^2 mean scaling is folded into the pairwise phase:
sim = (s_i*s_j)/N^4). pair_loss is symmetric, so sum over i<j equals
(sum over all i,j - sum over diagonal) / 2, with
    pair(i,j) = relu(d) - same*d,   d = sim - 0.5
    pair(i,i) = relu(0.5 - g_i^2) = -min(g_i^2 - 0.5, 0)

Perf notes:
- load DMAs alternate the two HWDGE rings (sync / scalar engines);
  free-axis reduction alternates DVE tensor_reduce (1x uop only) and
  ACT activation-Copy-with-accum so no engine gates the DMA stream.
- pairwise phase is DVE-only: the gathered [B] vector is broadcast
  across partitions with a step-0 DMA, the per-partition column scalars
  come from a strided DMA, and relu/sub/rowsum fuse into
  scalar_tensor_tensor with accum_out.
"""

import numpy as np

import concourse.bacc as bacc
import concourse.mybir as mybir
import concourse.tile as tile
from concourse.bass_utils import run_bass_kernel_spmd

N_CORES = 8
B = 256
N = 512
BS = B // N_CORES          # 32 graphs per core
NN = N * N                 # 262144 elements per graph
P = 128                    # SBUF partitions
FREE = NN // P             # 2048 f32 per partition per graph
MARGIN = 0.5
NUM_PAIRS = B * (B - 1) // 2
INV2 = 1.0 / (float(NN) * float(NN))   # folds the two mean divisions
KK = 0.5 / NUM_PAIRS

# knobs for test.py (harness never touches these)
TRACE = False
TRACE_DIR = None
TRACE_CORES = None
LAST_EXEC_NS = None
LAST_RESULTS = None

_CACHED_NC = None


def build_body(tc, loss_ap, graph_ap, labels_ap):
    """Emit the per-core program. graph_ap: [BS, N, N] f32 shard,
    labels_ap: [1, B] f32 full labels, loss_ap: [1, 1] f32 out."""
    nc = tc.nc
    f32 = mybir.dt.float32
    X = mybir.AxisListType.X
    ALU = mybir.AluOpType
    Copy = mybir.ActivationFunctionType.Copy

    # [BS, N, N] -> [BS, P, FREE]; per partition a contiguous 8 KiB run
    gview = graph_ap.rearrange("b n m -> b (n m)").rearrange(
        "b (p c) -> b p c", p=P
    )

    with (
        tc.tile_pool(name="io", bufs=8) as io_pool,
        tc.tile_pool(name="acc", bufs=1) as acc,
        tc.tile_pool(name="ps1", bufs=1, space="PSUM") as ps1,
        tc.tile_pool(name="dram", bufs=1, space="DRAM") as dram,
    ):
        S = acc.tile([P, BS], f32, tag="S")           # per-graph column sums
        ones_col = acc.tile([P, 1], f32, tag="ones_col")
        nc.vector.memset(ones_col[:], 1.0)

        # labels prep — independent of the graph data, runs during load
        lab_row = acc.tile([1, B], f32, tag="lab_row")
        nc.sync.dma_start(lab_row[:], labels_ap)
        lb = acc.tile([P, B], f32, tag="lb")
        nc.gpsimd.partition_broadcast(lb[:], lab_row[:])
        sames = []
        for c in range(2):
            lab_col = acc.tile([P, 1], f32, tag=f"lab_col{c}")
            nc.scalar.dma_start(lab_col[:], labels_ap[0, c * P : (c + 1) * P])
            same = acc.tile([P, B], f32, tag=f"same{c}")
            nc.vector.tensor_scalar(
                same[:], lb[:], lab_col[:], None, ALU.is_equal
            )
            sames.append(same)

        # ---- heavy phase: sum-pool each graph (DMA-bound) ----
        # all loads on the sync HWDGE ring (a second ring on the scalar
        # engine serializes behind its own ACTIVATE ops); the free-axis
        # reduce alternates DVE tensor_reduce and ACT activation-accum so
        # neither compute engine gates the DMA stream.
        # dummy 32-byte AllGather fired mid-load (issued after graph #18,
        # gated on columns 8:16 of S): re-synchronizes the 8 ranks well
        # before the real AllGather, whose entry barrier + ncfw wakeup then
        # cost ~6us instead of 20-35us. Runs on TOPSP; overlaps the load.
        warm_in = dram.tile([1, 8], f32, tag="warm_in")
        warm_out = dram.tile([N_CORES, 8], f32, tag="warm_out")

        for b in range(BS):
            t = io_pool.tile([P, FREE], f32, tag="gtile")
            nc.sync.dma_start(t[:], gview[b])
            if b % 2 == 0:
                nc.vector.reduce_sum(S[:, b : b + 1], t[:], axis=X)
            else:
                nc.scalar.activation(
                    t[:], t[:], Copy, accum_out=S[:, b : b + 1]
                )
            if b == 18:
                nc.sync.dma_start(warm_in[:], S[0:1, 8:16])
                nc.gpsimd.collective_compute(
                    "AllGather",
                    ALU.bypass,
                    replica_groups=[list(range(N_CORES))],
                    ins=[warm_in[:]],
                    outs=[warm_out[:]],
                )


        # cross-partition sum via PE: [1, BS] = ones.T @ S, split in halves
        # so the first matmul fires as soon as the first 16 graphs are done.
        H = BS // 2
        ps_g = ps1.tile([1, BS], f32, tag="ps_g")
        nc.tensor.matmul(ps_g[:, 0:H], ones_col[:], S[:, 0:H])
        nc.tensor.matmul(ps_g[:, H:BS], ones_col[:], S[:, H:BS])
        g_sb = acc.tile([1, BS], f32, tag="g_sb")
        nc.vector.tensor_copy(g_sb[:], ps_g[:])

        # ---- all-gather the [BS] raw sums -> [B] ----
        cc_in = dram.tile([1, BS], f32, tag="cc_in")
        cc_out = dram.tile([N_CORES, BS], f32, tag="cc_out")
        nc.sync.dma_start(cc_in[:], g_sb[:])
        nc.gpsimd.collective_compute(
            "AllGather",
            ALU.bypass,
            replica_groups=[list(range(N_CORES))],
            ins=[cc_in[:]],
            outs=[cc_out[:]],
        )

        # gathered raw sums: one small row DMA, then broadcast across
        # partitions on GpSimd (a step-0 broadcast DMA measured 5.7us)
        flat = cc_out[:].rearrange("r b -> (r b)")
        g_row = acc.tile([1, B], f32, tag="g_row")
        nc.sync.dma_start(g_row[:], flat[None, :])
        gb = acc.tile([P, B], f32, tag="gb")
        nc.gpsimd.partition_broadcast(gb[:], g_row[:])
        gcolk = []
        chunked = flat.rearrange("(c p) -> c p", c=2)
        for c in range(2):
            gcol = acc.tile([P, 1], f32, tag=f"gcol{c}")
            nc.scalar.dma_start(gcol[:], chunked[c][:, None])
            gk = acc.tile([P, 1], f32, tag=f"gcolk{c}")
            nc.vector.tensor_scalar(gk[:], gcol[:], INV2, None, ALU.mult)
            gcolk.append(gk)

        # ---- pairwise loss, two 128-row chunks, DVE only ----
        CS = acc.tile([P, 2], f32, tag="CS")
        for c in range(2):
            d = acc.tile([P, B], f32, tag=f"d{c}")      # s_i*s_j/NN^2 - 0.5
            nc.vector.tensor_scalar(
                d[:], gb[:], gcolk[c][:], -MARGIN, ALU.mult, op1=ALU.add
            )
            sd = acc.tile([P, B], f32, tag=f"sd{c}")    # same * d
            nc.vector.tensor_tensor(sd[:], sames[c][:], d[:], ALU.mult)
            pair = acc.tile([P, B], f32, tag=f"pair{c}")  # relu(d) - sd
            nc.vector.scalar_tensor_tensor(
                pair[:], d[:], 0.0, sd[:], ALU.max, ALU.subtract,
                accum_out=CS[:, c : c + 1],
            )

        # diagonal terms: -sum_diag*KK = sum(min(gg*INV2*KK - 0.5*KK, 0)).
        # The square and shift run on ACT (idle here), in parallel with the
        # DVE chunk chain: gg2 = (g*sqrt(INV2*KK))^2, dt = gg2 - 0.5*KK.
        sq = float(np.sqrt(INV2 * KK))
        gg2 = acc.tile([1, B], f32, tag="gg2")
        nc.scalar.activation(
            gg2[:], g_row[:], mybir.ActivationFunctionType.Square, scale=sq
        )
        dt = acc.tile([1, B], f32, tag="dt")
        nc.scalar.activation(dt[:], gg2[:], Copy, bias=-MARGIN * KK)
        du = acc.tile([1, B], f32, tag="du")
        dsumk = acc.tile([1, 1], f32, tag="dsumk")
        nc.vector.tensor_scalar(
            du[:], dt[:], 0.0, None, ALU.min, op1=ALU.add, accum_out=dsumk[:]
        )

        # total = sum all (i,j); loss = KK*total + dsumk
        ps_tot = ps1.tile([1, 2], f32, tag="ps_tot")
        nc.tensor.matmul(ps_tot[:], ones_col[:], CS[:])
        tk = acc.tile([1, 2], f32, tag="tk")
        totk = acc.tile([1, 1], f32, tag="totk")
        nc.vector.tensor_scalar(
            tk[:], ps_tot[:], KK, None, ALU.mult, op1=ALU.add,
            accum_out=totk[:],
        )
        res = acc.tile([1, 1], f32, tag="res")
        nc.vector.tensor_tensor(res[:], totk[:], dsumk[:], ALU.add)
        nc.sync.dma_start(loss_ap, res[:])


def _build():
    global _CACHED_NC
    if _CACHED_NC is not None:
        return _CACHED_NC
    nc = bacc.Bacc(
        "TRN2", target_bir_lowering=False, debug=False, num_devices=N_CORES
    )
    g_in = nc.dram_tensor(
        "graph", [BS, N, N], mybir.dt.float32, kind="ExternalInput"
    )
    lab_in = nc.dram_tensor(
        "labels_f32", [1, B], mybir.dt.float32, kind="ExternalInput"
    )
    out = nc.dram_tensor("loss", [1, 1], mybir.dt.float32, kind="ExternalOutput")
    with tile.TileContext(nc) as tc:
        build_body(tc, out.ap(), g_in.ap(), lab_in.ap())
    nc.compile()
    _CACHED_NC = nc
    return nc


def kernel(graph, labels):
    global LAST_EXEC_NS, LAST_RESULTS
    graph = np.ascontiguousarray(np.asarray(graph), dtype=np.float32)
    labels_f32 = np.asarray(labels).astype(np.float32).reshape(1, B)
    assert graph.shape == (B, N, N)

    nc = _build()
    in_maps = [
        {"graph": graph[c * BS : (c + 1) * BS], "labels_f32": labels_f32}
        for c in range(N_CORES)
    ]
    res = run_bass_kernel_spmd(
        nc,
        in_maps,
        core_ids=list(range(N_CORES)),
        trace=TRACE,
        tmpdir=TRACE_DIR,
        trace_cores=TRACE_CORES,
    )
    LAST_RESULTS = res
    LAST_EXEC_NS = res.exec_time_ns
    return np.asarray(res.results[0]["loss"][0, 0], dtype=np.float32)


# revision 28
# speedup vs baseline: 1.1170x; 1.1170x over previous
"""Bass/Trainium2 kernel for nn_ContrastiveAlignmentLoss.

reference math (B=256, N=512):
    global_sim = graph.mean(axis=(1, 2))                    # [B]
    sim        = outer(global_sim, global_sim)              # [B, B]
    same       = labels[:, None] == labels[None, :]
    pair_loss  = where(same, relu(0.5 - sim), relu(sim - 0.5))
    loss       = sum(triu(pair_loss, k=1)) / (B*(B-1)/2)

Distribution: data-parallel over B across 8 NeuronCores. Each core
sum-pools its 32 relation graphs (the memory-bound part: 32 MiB/core),
AllGathers the tiny [B] raw sums, and computes the pairwise loss
replicated (the 1/N^2 mean scaling is folded into the pairwise phase:
sim = (s_i*s_j)/N^4). pair_loss is symmetric, so sum over i<j equals
(sum over all i,j - sum over diagonal) / 2, with
    pair(i,j) = relu(d) - same*d,   d = sim - 0.5
    pair(i,i) = relu(0.5 - g_i^2) = -min(g_i^2 - 0.5, 0)

Perf notes:
- all load DMAs stream on the sync HWDGE ring; the free-axis reduction
  alternates DVE tensor_reduce (only has a 1x uop) and ACT
  activation-Copy-with-accum so no compute engine gates the DMA stream
  (~300 GB/s/core, the HBM-pair contention wall).
- a 32-byte warm-up AllGather fired mid-load re-synchronizes the ranks
  so the real AllGather's ncfw wakeup + entry barrier cost ~6us instead
  of 20-35us.
- pairwise phase is DVE-only: the gathered [B] vector is broadcast
  across partitions on GpSimd, the per-partition column scalars come
  from a strided DMA, and relu/sub/rowsum fuse into
  scalar_tensor_tensor with accum_out.
"""

import numpy as np

import concourse.bacc as bacc
import concourse.mybir as mybir
import concourse.tile as tile
from concourse.bass_utils import run_bass_kernel_spmd

N_CORES = 8
B = 256
N = 512
BS = B // N_CORES          # 32 graphs per core
NN = N * N                 # 262144 elements per graph
P = 128                    # SBUF partitions
FREE = NN // P             # 2048 f32 per partition per graph
MARGIN = 0.5
NUM_PAIRS = B * (B - 1) // 2
INV2 = 1.0 / (float(NN) * float(NN))   # folds the two mean divisions
KK = 0.5 / NUM_PAIRS

# knobs for test.py (harness never touches these)
TRACE = False
TRACE_DIR = None
TRACE_CORES = None
LAST_EXEC_NS = None
LAST_RESULTS = None

_CACHED_NC = None


def build_body(tc, loss_ap, graph_ap, labels_ap):
    """Emit the per-core program. graph_ap: [BS, N, N] f32 shard,
    labels_ap: [1, B] f32 full labels, loss_ap: [1, 1] f32 out."""
    nc = tc.nc
    f32 = mybir.dt.float32
    X = mybir.AxisListType.X
    ALU = mybir.AluOpType
    Copy = mybir.ActivationFunctionType.Copy

    # [BS, N, N] -> [BS, P, FREE]; per partition a contiguous 8 KiB run
    gview = graph_ap.rearrange("b n m -> b (n m)").rearrange(
        "b (p c) -> b p c", p=P
    )

    with (
        tc.tile_pool(name="io", bufs=8) as io_pool,
        tc.tile_pool(name="acc", bufs=1) as acc,
        tc.tile_pool(name="ps1", bufs=1, space="PSUM") as ps1,
        tc.tile_pool(name="dram", bufs=1, space="DRAM") as dram,
    ):
        S = acc.tile([P, BS], f32, tag="S")           # per-graph column sums
        ones_col = acc.tile([P, 1], f32, tag="ones_col")
        nc.vector.memset(ones_col[:], 1.0)

        # labels prep — independent of the graph data, runs during load
        lab_row = acc.tile([1, B], f32, tag="lab_row")
        nc.sync.dma_start(lab_row[:], labels_ap)
        lb = acc.tile([P, B], f32, tag="lb")
        nc.gpsimd.partition_broadcast(lb[:], lab_row[:])
        sames = []
        for c in range(2):
            lab_col = acc.tile([P, 1], f32, tag=f"lab_col{c}")
            nc.scalar.dma_start(lab_col[:], labels_ap[0, c * P : (c + 1) * P])
            same = acc.tile([P, B], f32, tag=f"same{c}")
            nc.vector.tensor_scalar(
                same[:], lb[:], lab_col[:], None, ALU.is_equal
            )
            sames.append(same)

        # ---- heavy phase: sum-pool each graph (DMA-bound) ----
        # all loads on the sync HWDGE ring (a second ring on the scalar
        # engine serializes behind its own ACTIVATE ops); the free-axis
        # reduce alternates DVE tensor_reduce and ACT activation-accum so
        # neither compute engine gates the DMA stream.
        # dummy 32-byte AllGather fired mid-load (issued after graph #18,
        # gated on columns 8:16 of S): re-synchronizes the 8 ranks well
        # before the real AllGather, whose entry barrier + ncfw wakeup then
        # cost ~6us instead of 20-35us. Runs on TOPSP; overlaps the load.
        warm_in = dram.tile([1, 8], f32, tag="warm_in")
        warm_out = dram.tile([N_CORES, 8], f32, tag="warm_out")

        for b in range(BS):
            t = io_pool.tile([P, FREE], f32, tag="gtile")
            nc.sync.dma_start(t[:], gview[b])
            if b % 2 == 0:
                nc.vector.reduce_sum(S[:, b : b + 1], t[:], axis=X)
            else:
                nc.scalar.activation(
                    t[:], t[:], Copy, accum_out=S[:, b : b + 1]
                )
            if b == 18:
                nc.sync.dma_start(warm_in[:], S[0:1, 8:16])
                nc.gpsimd.collective_compute(
                    "AllGather",
                    ALU.bypass,
                    replica_groups=[list(range(N_CORES))],
                    ins=[warm_in[:]],
                    outs=[warm_out[:]],
                )

        # cross-partition sum via PE: [1, BS] = ones.T @ S, split in halves
        # so the first matmul fires as soon as the first 16 graphs are done.
        H = BS // 2
        ps_g = ps1.tile([1, BS], f32, tag="ps_g")
        nc.tensor.matmul(ps_g[:, 0:H], ones_col[:], S[:, 0:H])
        nc.tensor.matmul(ps_g[:, H:BS], ones_col[:], S[:, H:BS])
        g_sb = acc.tile([1, BS], f32, tag="g_sb")
        nc.vector.tensor_copy(g_sb[:], ps_g[:])

        # ---- all-gather the [BS] raw sums -> [B] ----
        cc_in = dram.tile([1, BS], f32, tag="cc_in")
        cc_out = dram.tile([N_CORES, BS], f32, tag="cc_out")
        nc.sync.dma_start(cc_in[:], g_sb[:])
        nc.gpsimd.collective_compute(
            "AllGather",
            ALU.bypass,
            replica_groups=[list(range(N_CORES))],
            ins=[cc_in[:]],
            outs=[cc_out[:]],
        )

        # gathered raw sums: one small row DMA, then broadcast across
        # partitions on GpSimd (a step-0 broadcast DMA measured 5.7us)
        flat = cc_out[:].rearrange("r b -> (r b)")
        g_row = acc.tile([1, B], f32, tag="g_row")
        nc.sync.dma_start(g_row[:], flat[None, :])
        gb = acc.tile([P, B], f32, tag="gb")
        nc.gpsimd.partition_broadcast(gb[:], g_row[:])
        gcolk = []
        chunked = flat.rearrange("(c p) -> c p", c=2)
        for c in range(2):
            gcol = acc.tile([P, 1], f32, tag=f"gcol{c}")
            nc.scalar.dma_start(gcol[:], chunked[c][:, None])
            gk = acc.tile([P, 1], f32, tag=f"gcolk{c}")
            nc.vector.tensor_scalar(gk[:], gcol[:], INV2, None, ALU.mult)
            gcolk.append(gk)

        # ---- pairwise loss, two 128-row chunks, DVE only ----
        CS = acc.tile([P, 2], f32, tag="CS")
        for c in range(2):
            d = acc.tile([P, B], f32, tag=f"d{c}")      # s_i*s_j/NN^2 - 0.5
            nc.vector.tensor_scalar(
                d[:], gb[:], gcolk[c][:], -MARGIN, ALU.mult, op1=ALU.add
            )
            sd = acc.tile([P, B], f32, tag=f"sd{c}")    # same * d
            nc.vector.tensor_tensor(sd[:], sames[c][:], d[:], ALU.mult)
            pair = acc.tile([P, B], f32, tag=f"pair{c}")  # relu(d) - sd
            nc.vector.scalar_tensor_tensor(
                pair[:], d[:], 0.0, sd[:], ALU.max, ALU.subtract,
                accum_out=CS[:, c : c + 1],
            )

        # diagonal terms: -sum_diag*KK = sum(min(gg*INV2*KK - 0.5*KK, 0)).
        # The square and shift run on ACT (idle here), in parallel with the
        # DVE chunk chain: gg2 = (g*sqrt(INV2*KK))^2, dt = gg2 - 0.5*KK.
        sq = float(np.sqrt(INV2 * KK))
        gg2 = acc.tile([1, B], f32, tag="gg2")
        nc.scalar.activation(
            gg2[:], g_row[:], mybir.ActivationFunctionType.Square, scale=sq
        )
        dt = acc.tile([1, B], f32, tag="dt")
        nc.scalar.activation(dt[:], gg2[:], Copy, bias=-MARGIN * KK)
        du = acc.tile([1, B], f32, tag="du")
        dsumk = acc.tile([1, 1], f32, tag="dsumk")
        nc.vector.tensor_scalar(
            du[:], dt[:], 0.0, None, ALU.min, op1=ALU.add, accum_out=dsumk[:]
        )

        # total = sum all (i,j); loss = KK*total + dsumk
        ps_tot = ps1.tile([1, 2], f32, tag="ps_tot")
        nc.tensor.matmul(ps_tot[:], ones_col[:], CS[:])
        tk = acc.tile([1, 2], f32, tag="tk")
        totk = acc.tile([1, 1], f32, tag="totk")
        nc.vector.tensor_scalar(
            tk[:], ps_tot[:], KK, None, ALU.mult, op1=ALU.add,
            accum_out=totk[:],
        )
        res = acc.tile([1, 1], f32, tag="res")
        nc.vector.tensor_tensor(res[:], totk[:], dsumk[:], ALU.add)
        nc.sync.dma_start(loss_ap, res[:])


def _build():
    global _CACHED_NC
    if _CACHED_NC is not None:
        return _CACHED_NC
    nc = bacc.Bacc(
        "TRN2", target_bir_lowering=False, debug=False, num_devices=N_CORES
    )
    g_in = nc.dram_tensor(
        "graph", [BS, N, N], mybir.dt.float32, kind="ExternalInput"
    )
    lab_in = nc.dram_tensor(
        "labels_f32", [1, B], mybir.dt.float32, kind="ExternalInput"
    )
    out = nc.dram_tensor("loss", [1, 1], mybir.dt.float32, kind="ExternalOutput")
    with tile.TileContext(nc) as tc:
        build_body(tc, out.ap(), g_in.ap(), lab_in.ap())
    nc.compile()
    _CACHED_NC = nc
    return nc


def kernel(graph, labels):
    global LAST_EXEC_NS, LAST_RESULTS
    graph = np.ascontiguousarray(np.asarray(graph), dtype=np.float32)
    labels_f32 = np.asarray(labels).astype(np.float32).reshape(1, B)
    assert graph.shape == (B, N, N)

    nc = _build()
    in_maps = [
        {"graph": graph[c * BS : (c + 1) * BS], "labels_f32": labels_f32}
        for c in range(N_CORES)
    ]
    res = run_bass_kernel_spmd(
        nc,
        in_maps,
        core_ids=list(range(N_CORES)),
        trace=TRACE,
        tmpdir=TRACE_DIR,
        trace_cores=TRACE_CORES,
    )
    LAST_RESULTS = res
    LAST_EXEC_NS = res.exec_time_ns
    return np.asarray(res.results[0]["loss"][0, 0], dtype=np.float32)

